# revision 1
# baseline (speedup 1.0000x reference)
"""DGN agent (2-layer graph attention) Trainium2 Bass kernel, v2 (bf16).

Dataflow per 128-row group (4 batches x 32 agents, block-diagonal):
  - Activations feature-major [128 feat, rows]; all matmuls bf16 (4x PE
    throughput vs fp32), fp32 PSUM accumulation.
  - Scores ROW-major s[i,j]: additive mask (0 / -1024) DMA-prefilled into
    the PSUM bank, score matmuls accumulate onto it; exp(s-1024)==0 exactly.
  - ACT exp emits per-partition row-sums via accum_out (masked already);
    reciprocal is a tiny [128,4] DVE op; normalization is a per-partition
    bf16 tensor_scalar (DVE 2x mode).
  - att transposed with the DVE 32x32 stream transpose (block-diag => block
    transpose IS the transpose).
  - Out-projection refactor: ow.T(att@v) = (v@ow).T attT. u = v@ow runs as
    group matmuls (lhsT = v_fm slice); h' = u.T @ attT lands feature-major
    with per-partition relu+bias. No rank-1 bias matmuls, no ao copy.
  - Weights packed in one bf16 blob, biases in one fp32 blob (fewer
    dispatch handles + single DMA each).
"""

import numpy as np

import concourse.bass as bass
import concourse.mybir as mybir
import concourse.tile as tile
from concourse import bacc
from concourse.bass_utils import run_bass_kernel_spmd

F32 = mybir.dt.float32
BF16 = mybir.dt.bfloat16
AX = mybir.AxisListType
OP = mybir.AluOpType
AF = mybir.ActivationFunctionType

B, N, DIN, H, A = 4096, 32, 256, 128, 32
NCORES = 8
BC = B // NCORES          # batches per core
R = BC * N                # rows per core (16384)
MASK_C = 1024.0

# wblob column offsets (bf16 [128, 1312])
WOFF = {"enc": 0, "a1_q": 256, "a1_k": 384, "a1_v": 512, "a1_o": 640,
        "a2_q": 768, "a2_k": 896, "a2_v": 1024, "a2_o": 1152, "qw": 1280}
WCOLS = 1312
# bblob column index (fp32 [128, 10])
BOFF = {"enc": 0, "a1_q": 1, "a1_k": 2, "a1_v": 3, "a1_o": 4,
        "a2_q": 5, "a2_k": 6, "a2_v": 7, "a2_o": 8, "qb": 9}


def build_program(n_rows, rt=2048, n_cores=NCORES, pbufs=(3, 2, 2, 1),
                  sbufs=4):
    assert n_rows % rt == 0 and rt % 512 == 0
    n_rt = n_rows // rt
    gpt = rt // 128           # groups per row tile
    ngg = gpt // 4            # psum-bank quads per row tile
    ck = rt // 512            # 512-col chunks per row tile

    nc = bacc.Bacc("TRN2", target_bir_lowering=False, debug=False,
                   num_devices=n_cores)

    xt_d = nc.dram_tensor("xt", [DIN, n_rows], BF16, kind="ExternalInput")
    mc_d = nc.dram_tensor("metc", [128, n_rows // 128, 128], BF16,
                          kind="ExternalInput")
    wb_d = nc.dram_tensor("wblob", [128, WCOLS], BF16, kind="ExternalInput")
    bb_d = nc.dram_tensor("bblob", [128, len(BOFF)], F32,
                          kind="ExternalInput")
    y_d = nc.dram_tensor("y", [A, n_rows], F32, kind="ExternalOutput")

    with tile.TileContext(nc) as tc:
        with (
            tc.tile_pool(name="singles", bufs=1) as singles,
            tc.tile_pool(name="xt", bufs=2) as xt_pool,
            tc.tile_pool(name="met", bufs=2) as met_pool,
            tc.tile_pool(name="acts", bufs=3) as act_pool,
            tc.tile_pool(name="qkv", bufs=2) as qkv_pool,
            tc.tile_pool(name="sm", bufs=sbufs) as sm_pool,
            tc.tile_pool(name="out", bufs=2) as out_pool,
            tc.tile_pool(name="pproj", bufs=pbufs[0], space="PSUM") as pproj,
            tc.tile_pool(name="psc", bufs=pbufs[1], space="PSUM") as psc_pool,
            tc.tile_pool(name="pu", bufs=pbufs[2], space="PSUM") as pu_pool,
            tc.tile_pool(name="ph", bufs=pbufs[3], space="PSUM") as ph_pool,
        ):
            wb = singles.tile([128, WCOLS], BF16, tag="wb")
            nc.sync.dma_start(out=wb, in_=wb_d.ap())
            bb = singles.tile([128, len(BOFF)], F32, tag="bb")
            nc.sync.dma_start(out=bb, in_=bb_d.ap())

            def W(nm, w=H):
                return wb[:, WOFF[nm]:WOFF[nm] + w]

            def Bi(nm):
                return bb[:, BOFF[nm]:BOFF[nm] + 1]

            for irt in range(n_rt):
                r0 = irt * rt
                xt_sb = xt_pool.tile([128, 2, rt], BF16, tag="xt")
                nc.sync.dma_start(
                    out=xt_sb,
                    in_=xt_d.ap().rearrange("(c k) r -> k c r", c=2)
                    [:, :, r0:r0 + rt])
                mc_sb = met_pool.tile([128, gpt, 128], BF16, tag="mc")
                nc.sync.dma_start(
                    out=mc_sb,
                    in_=mc_d.ap()[:, r0 // 128:r0 // 128 + gpt, :])

                # ---- encoder ----------------------------------------
                act = act_pool.tile([128, rt], BF16, tag="act")
                for c in range(ck):
                    sl = bass.ts(c, 512)
                    ps = pproj.tile([128, 512], F32, tag="proj")
                    nc.tensor.matmul(ps, W("enc", 256).rearrange(
                        "p (c h) -> p c h", c=2)[:, 0, :],
                        xt_sb[:, 0, sl], start=True, stop=False)
                    nc.tensor.matmul(ps, W("enc", 256).rearrange(
                        "p (c h) -> p c h", c=2)[:, 1, :],
                        xt_sb[:, 1, sl], start=False, stop=True)
                    nc.vector.tensor_scalar(out=act[:, sl], in0=ps,
                                            scalar1=Bi("enc"), scalar2=0.0,
                                            op0=OP.add, op1=OP.max)

                for lname in ("a1", "a2"):
                    # ---- q,k,v projections (feature-major) ----------
                    q_sb = qkv_pool.tile([128, rt], BF16, tag="q")
                    k_sb = qkv_pool.tile([128, rt], BF16, tag="k")
                    v_sb = qkv_pool.tile([128, rt], BF16, tag="v")
                    for c in range(ck):
                        sl = bass.ts(c, 512)
                        psq = pproj.tile([128, 512], F32, tag="proj")
                        nc.tensor.matmul(psq, W(f"{lname}_q"), act[:, sl],
                                         start=True, stop=True)
                        nc.scalar.activation(out=q_sb[:, sl], in_=psq,
                                             func=AF.Relu,
                                             bias=Bi(f"{lname}_q"), scale=1.0)
                        psk = pproj.tile([128, 512], F32, tag="proj")
                        nc.tensor.matmul(psk, W(f"{lname}_k"), act[:, sl],
                                         start=True, stop=True)
                        nc.vector.tensor_scalar(
                            out=k_sb[:, sl], in0=psk,
                            scalar1=Bi(f"{lname}_k"), scalar2=0.0,
                            op0=OP.add, op1=OP.max)
                        psv = pproj.tile([128, 512], F32, tag="proj")
                        nc.tensor.matmul(psv, W(f"{lname}_v"), act[:, sl],
                                         start=True, stop=True)
                        nc.scalar.activation(out=v_sb[:, sl], in_=psv,
                                             func=AF.Relu,
                                             bias=Bi(f"{lname}_v"), scale=1.0)

                    nact = act_pool.tile([128, rt], BF16, tag="act")
                    for gg in range(ngg):
                        g0 = gg * 4
                        sc = psc_pool.tile([128, 4, 128], F32, tag="sc")
                        for gi in range(4):
                            gsl = bass.ts(g0 + gi, 128)
                            nc.tensor.matmul(sc[:, gi, :], q_sb[:, gsl],
                                             k_sb[:, gsl], start=True,
                                             stop=True)
                        # exp (one op), then mask-multiply + masked row-sums
                        # in one bf16 DVE pass per group (accum_out)
                        eraw = sm_pool.tile([128, 4, 128], BF16, tag="eraw")
                        nc.scalar.activation(out=eraw, in_=sc, func=AF.Exp,
                                             bias=0.0, scale=1.0)
                        e_sb = sm_pool.tile([128, 4, 128], BF16, tag="e")
                        rs = sm_pool.tile([128, 4], F32, tag="rs")
                        for gi in range(4):
                            nc.vector.scalar_tensor_tensor(
                                out=e_sb[:, gi, :], in0=eraw[:, gi, :],
                                scalar=1.0, in1=mc_sb[:, g0 + gi, :],
                                op0=OP.bypass, op1=OP.mult,
                                accum_out=rs[:, gi:gi + 1])
                        rr = sm_pool.tile([128, 4], F32, tag="rr")
                        nc.vector.reciprocal(out=rr, in_=rs)
                        att = sm_pool.tile([128, 4, 128], BF16, tag="att")
                        for gi in range(4):
                            nc.gpsimd.tensor_scalar(
                                out=att[:, gi, :], in0=e_sb[:, gi, :],
                                scalar1=rr[:, gi:gi + 1], scalar2=None,
                                op0=OP.mult)
                        attT = sm_pool.tile([128, 4, 128], BF16, tag="attT")
                        nc.vector.transpose(
                            out=attT.rearrange("p g w -> p (g w)"),
                            in_=att.rearrange("p g w -> p (g w)"))
                        # u = v @ ow  (row-major), then h' = u.T @ attT
                        pu = pu_pool.tile([128, 4, 128], F32, tag="u")
                        for gi in range(4):
                            gsl = bass.ts(g0 + gi, 128)
                            nc.tensor.matmul(pu[:, gi, :], v_sb[:, gsl],
                                             W(f"{lname}_o"), start=True,
                                             stop=True)
                        u_sb = sm_pool.tile([128, 4, 128], BF16, tag="u_sb")
                        nc.vector.tensor_copy(out=u_sb, in_=pu)
                        ph = ph_pool.tile([128, 4, 128], F32, tag="h")
                        for gi in range(4):
                            nc.tensor.matmul(ph[:, gi, :], u_sb[:, gi, :],
                                             attT[:, gi, :], start=True,
                                             stop=True)
                        nc.scalar.activation(
                            out=nact[:, bass.ts(gg, 512)],
                            in_=ph.rearrange("p g w -> p (g w)"),
                            func=AF.Relu, bias=Bi(f"{lname}_o"), scale=1.0)
                    act = nact

                # ---- final linear: chunks packed 4x on partitions ----
                for c0 in range(0, ck, 4):
                    nq = min(4, ck - c0)
                    m = A * nq
                    ps = pproj.tile([128, 512], F32, tag="proj")
                    for ci in range(nq):
                        c = c0 + ci
                        nc.tensor.matmul(ps[A * ci:A * (ci + 1), :],
                                         W("qw", A), act[:, bass.ts(c, 512)],
                                         start=True, stop=True,
                                         skip_group_check=True,
                                         tile_position=(0, A * ci))
                    o_sb = out_pool.tile([128, 512], F32, tag="o")
                    nc.vector.tensor_scalar(out=o_sb[:m, :], in0=ps[:m, :],
                                            scalar1=Bi("qb")[:m, :],
                                            scalar2=None, op0=OP.add)
                    for ci in range(nq):
                        c = c0 + ci
                        nc.sync.dma_start(
                            out=y_d.ap()[:, r0 + 512 * c:r0 + 512 * (c + 1)],
                            in_=o_sb[A * ci:A * (ci + 1), :])

    nc.compile()
    return nc


def _bf16(a):
    import ml_dtypes
    return np.asarray(a, np.float32).astype(ml_dtypes.bfloat16)


def pack_weights(inputs):
    """Build the bf16 weight blob [128, WCOLS] and fp32 bias blob."""
    wb = np.zeros((128, WCOLS), np.float32)
    ew = np.asarray(inputs["enc_w"], np.float32)          # [256, 128]
    wb[:, 0:128] = ew[:128]
    wb[:, 128:256] = ew[128:]
    for l in ("a1", "a2"):
        for p in ("q", "k", "v", "o"):
            wb[:, WOFF[f"{l}_{p}"]:WOFF[f"{l}_{p}"] + H] = np.asarray(
                inputs[f"{l}_{p}w"], np.float32)
    wb[:, WOFF["qw"]:WOFF["qw"] + A] = np.asarray(inputs["q_w"], np.float32)
    bbl = np.zeros((128, len(BOFF)), np.float32)
    bbl[:, 0] = inputs["enc_b"]
    for l in ("a1", "a2"):
        for i, p in enumerate(("q", "k", "v", "o")):
            bbl[:, BOFF[f"{l}_{p}"]] = inputs[f"{l}_{p}b"]
    bbl[:, BOFF["qb"]] = np.tile(np.asarray(inputs["q_b"], np.float32), 4)
    return _bf16(wb), bbl


def prep_inputs_core(x_c, mask_c):
    """Per-core: x -> bf16 [DIN, rows]; mask -> (1-met) bf16 block tiles."""
    rows = x_c.shape[0] * N
    xt = _bf16(np.ascontiguousarray(
        np.asarray(x_c, np.float32).reshape(rows, DIN).T))
    ng = x_c.shape[0] // 4
    m4 = np.asarray(mask_c, np.float32).reshape(ng, 4, N, N)
    met = np.zeros((ng, 128, 128), np.float32)
    for b in range(4):
        met[:, 32 * b:32 * b + 32, 32 * b:32 * b + 32] = m4[:, b]
    mc = np.ascontiguousarray(met.transpose(1, 0, 2))      # [128, G, 128]
    return xt, _bf16(mc)


_CACHE = {}


def build_in_maps(inputs):
    x, mask = inputs["x"], inputs["mask"]
    wb, bbl = pack_weights(inputs)
    in_maps = []
    for c in range(NCORES):
        xt, mc = prep_inputs_core(x[c * BC:(c + 1) * BC],
                                  mask[c * BC:(c + 1) * BC])
        in_maps.append({"xt": xt, "metc": mc, "wblob": wb, "bblob": bbl})
    return in_maps


def kernel(**inputs):
    if "nc" not in _CACHE:
        _CACHE["nc"] = build_program(R)
    res = run_bass_kernel_spmd(_CACHE["nc"], build_in_maps(inputs),
                               core_ids=list(range(NCORES)))
    outs = [r["y"].T.reshape(BC, N, A) for r in res.results]
    return np.concatenate(outs, axis=0).astype(np.float32)



# revision 4
# speedup vs baseline: 1.2775x; 1.2775x over previous
"""DGN agent (2-layer graph attention) Trainium2 Bass kernel, v2 (bf16).

Dataflow per 128-row group (4 batches x 32 agents, block-diagonal):
  - Activations feature-major [128 feat, rows]; all matmuls bf16 (4x PE
    throughput vs fp32), fp32 PSUM accumulation.
  - Scores ROW-major s[i,j]: additive mask (0 / -1024) DMA-prefilled into
    the PSUM bank, score matmuls accumulate onto it; exp(s-1024)==0 exactly.
  - ACT exp emits per-partition row-sums via accum_out (masked already);
    reciprocal is a tiny [128,4] DVE op; normalization is a per-partition
    bf16 tensor_scalar (DVE 2x mode).
  - att transposed with the DVE 32x32 stream transpose (block-diag => block
    transpose IS the transpose).
  - Out-projection refactor: ow.T(att@v) = (v@ow).T attT. u = v@ow runs as
    group matmuls (lhsT = v_fm slice); h' = u.T @ attT lands feature-major
    with per-partition relu+bias. No rank-1 bias matmuls, no ao copy.
  - Weights packed in one bf16 blob, biases in one fp32 blob (fewer
    dispatch handles + single DMA each).
"""

import numpy as np

import concourse.bass as bass
import concourse.mybir as mybir
import concourse.tile as tile
from concourse import bacc
from concourse.bass_utils import run_bass_kernel_spmd

F32 = mybir.dt.float32
BF16 = mybir.dt.bfloat16
AX = mybir.AxisListType
OP = mybir.AluOpType
AF = mybir.ActivationFunctionType

B, N, DIN, H, A = 4096, 32, 256, 128, 32
NCORES = 8
BC = B // NCORES          # batches per core
R = BC * N                # rows per core (16384)
MASK_C = 1024.0

# wblob column offsets (bf16 [128, 1312])
WOFF = {"enc": 0, "a1_q": 256, "a1_k": 384, "a1_v": 512, "a1_o": 640,
        "a2_q": 768, "a2_k": 896, "a2_v": 1024, "a2_o": 1152, "qw": 1280}
WCOLS = 1312
# bblob column index (fp32 [128, 10])
BOFF = {"enc": 0, "a1_q": 1, "a1_k": 2, "a1_v": 3, "a1_o": 4,
        "a2_q": 5, "a2_k": 6, "a2_v": 7, "a2_o": 8, "qb": 9}


def build_program(n_rows, rt=2048, n_cores=NCORES, pbufs=(2, 2, 2, 2),
                  sbufs=4):
    assert n_rows % rt == 0 and rt % 512 == 0
    n_rt = n_rows // rt
    gpt = rt // 128           # groups per row tile
    ngg = gpt // 4            # psum-bank quads per row tile
    ck = rt // 512            # 512-col chunks per row tile

    nc = bacc.Bacc("TRN2", target_bir_lowering=False, debug=False,
                   num_devices=n_cores)

    xt_d = nc.dram_tensor("xt", [DIN, n_rows], BF16, kind="ExternalInput")
    mc_d = nc.dram_tensor("metc", [128, n_rows // 128, 128], BF16,
                          kind="ExternalInput")
    wb_d = nc.dram_tensor("wblob", [128, WCOLS], BF16, kind="ExternalInput")
    bb_d = nc.dram_tensor("bblob", [128, len(BOFF)], F32,
                          kind="ExternalInput")
    y_d = nc.dram_tensor("y", [A, n_rows], F32, kind="ExternalOutput")

    with tile.TileContext(nc) as tc:
        with (
            tc.tile_pool(name="singles", bufs=1) as singles,
            tc.tile_pool(name="xt", bufs=2) as xt_pool,
            tc.tile_pool(name="met", bufs=2) as met_pool,
            tc.tile_pool(name="acts", bufs=3) as act_pool,
            tc.tile_pool(name="qkv", bufs=2) as qkv_pool,
            tc.tile_pool(name="sm", bufs=sbufs) as sm_pool,
            tc.tile_pool(name="out", bufs=2) as out_pool,
            tc.tile_pool(name="pproj", bufs=pbufs[0], space="PSUM") as pproj,
            tc.tile_pool(name="psc", bufs=pbufs[1], space="PSUM") as psc_pool,
            tc.tile_pool(name="pu", bufs=pbufs[2], space="PSUM") as pu_pool,
            tc.tile_pool(name="ph", bufs=pbufs[3], space="PSUM") as ph_pool,
        ):
            wb = singles.tile([128, WCOLS], BF16, tag="wb")
            nc.sync.dma_start(out=wb, in_=wb_d.ap())
            bb = singles.tile([128, len(BOFF)], F32, tag="bb")
            nc.sync.dma_start(out=bb, in_=bb_d.ap())

            def W(nm, w=H):
                return wb[:, WOFF[nm]:WOFF[nm] + w]

            def Bi(nm):
                return bb[:, BOFF[nm]:BOFF[nm] + 1]

            for irt in range(n_rt):
                r0 = irt * rt
                xt_sb = xt_pool.tile([128, 2, rt], BF16, tag="xt")
                nc.sync.dma_start(
                    out=xt_sb,
                    in_=xt_d.ap().rearrange("(c k) r -> k c r", c=2)
                    [:, :, r0:r0 + rt])
                mc_sb = met_pool.tile([128, gpt, 128], BF16, tag="mc")
                nc.sync.dma_start(
                    out=mc_sb,
                    in_=mc_d.ap()[:, r0 // 128:r0 // 128 + gpt, :])

                # ---- encoder ----------------------------------------
                act = act_pool.tile([128, rt], BF16, tag="act")
                for c in range(ck):
                    sl = bass.ts(c, 512)
                    ps = pproj.tile([128, 512], F32, tag="proj")
                    nc.tensor.matmul(ps, W("enc", 256).rearrange(
                        "p (c h) -> p c h", c=2)[:, 0, :],
                        xt_sb[:, 0, sl], start=True, stop=False)
                    nc.tensor.matmul(ps, W("enc", 256).rearrange(
                        "p (c h) -> p c h", c=2)[:, 1, :],
                        xt_sb[:, 1, sl], start=False, stop=True)
                    nc.vector.tensor_scalar(out=act[:, sl], in0=ps,
                                            scalar1=Bi("enc"), scalar2=0.0,
                                            op0=OP.add, op1=OP.max)

                for lname in ("a1", "a2"):
                    # ---- q,k,v projections (feature-major) ----------
                    q_sb = qkv_pool.tile([128, rt], BF16, tag="q")
                    k_sb = qkv_pool.tile([128, rt], BF16, tag="k")
                    v_sb = qkv_pool.tile([128, rt], BF16, tag="v")
                    for c in range(ck):
                        sl = bass.ts(c, 512)
                        psq = pproj.tile([128, 512], F32, tag="proj")
                        nc.tensor.matmul(psq, W(f"{lname}_q"), act[:, sl],
                                         start=True, stop=True)
                        nc.scalar.activation(out=q_sb[:, sl], in_=psq,
                                             func=AF.Relu,
                                             bias=Bi(f"{lname}_q"), scale=1.0)
                        psk = pproj.tile([128, 512], F32, tag="proj")
                        nc.tensor.matmul(psk, W(f"{lname}_k"), act[:, sl],
                                         start=True, stop=True)
                        nc.vector.tensor_scalar(
                            out=k_sb[:, sl], in0=psk,
                            scalar1=Bi(f"{lname}_k"), scalar2=0.0,
                            op0=OP.add, op1=OP.max)
                        psv = pproj.tile([128, 512], F32, tag="proj")
                        nc.tensor.matmul(psv, W(f"{lname}_v"), act[:, sl],
                                         start=True, stop=True)
                        nc.scalar.activation(out=v_sb[:, sl], in_=psv,
                                             func=AF.Relu,
                                             bias=Bi(f"{lname}_v"), scale=1.0)

                    nact = act_pool.tile([128, rt], BF16, tag="act")
                    for gg in range(ngg):
                        g0 = gg * 4
                        sc = psc_pool.tile([128, 4, 128], F32, tag="sc")
                        for gi in range(4):
                            gsl = bass.ts(g0 + gi, 128)
                            nc.tensor.matmul(sc[:, gi, :], q_sb[:, gsl],
                                             k_sb[:, gsl], start=True,
                                             stop=True)
                        # exp (one op), then one big mask-multiply, then a
                        # segmented row-sum reduce (all bf16 DVE 2x mode)
                        eraw = sm_pool.tile([128, 4, 128], BF16, tag="eraw")
                        nc.scalar.activation(out=eraw, in_=sc, func=AF.Exp,
                                             bias=0.0, scale=1.0)
                        e_sb = sm_pool.tile([128, 4, 128], BF16, tag="e")
                        nc.vector.tensor_tensor(
                            out=e_sb, in0=eraw,
                            in1=mc_sb[:, g0:g0 + 4, :], op=OP.mult)
                        rs = sm_pool.tile([128, 4], F32, tag="rs")
                        nc.vector.tensor_reduce(out=rs, in_=e_sb, axis=AX.X,
                                                op=OP.add)
                        rr = sm_pool.tile([128, 4], F32, tag="rr")
                        nc.vector.reciprocal(out=rr, in_=rs)
                        att = sm_pool.tile([128, 4, 128], BF16, tag="att")
                        for gi in range(4):
                            nc.vector.tensor_scalar(
                                out=att[:, gi, :], in0=e_sb[:, gi, :],
                                scalar1=rr[:, gi:gi + 1], scalar2=None,
                                op0=OP.mult)
                        attT = sm_pool.tile([128, 4, 128], BF16, tag="attT")
                        nc.vector.transpose(
                            out=attT.rearrange("p g w -> p (g w)"),
                            in_=att.rearrange("p g w -> p (g w)"))
                        # u = v @ ow  (row-major), then h' = u.T @ attT
                        pu = pu_pool.tile([128, 4, 128], F32, tag="u")
                        for gi in range(4):
                            gsl = bass.ts(g0 + gi, 128)
                            nc.tensor.matmul(pu[:, gi, :], v_sb[:, gsl],
                                             W(f"{lname}_o"), start=True,
                                             stop=True)
                        u_sb = sm_pool.tile([128, 4, 128], BF16, tag="u_sb")
                        nc.scalar.copy(out=u_sb, in_=pu)
                        ph = ph_pool.tile([128, 4, 128], F32, tag="h")
                        for gi in range(4):
                            nc.tensor.matmul(ph[:, gi, :], u_sb[:, gi, :],
                                             attT[:, gi, :], start=True,
                                             stop=True)
                        nc.scalar.activation(
                            out=nact[:, bass.ts(gg, 512)],
                            in_=ph.rearrange("p g w -> p (g w)"),
                            func=AF.Relu, bias=Bi(f"{lname}_o"), scale=1.0)
                    act = nact

                # ---- final linear: chunks packed 4x on partitions ----
                for c0 in range(0, ck, 4):
                    nq = min(4, ck - c0)
                    m = A * nq
                    ps = pproj.tile([128, 512], F32, tag="proj")
                    for ci in range(nq):
                        c = c0 + ci
                        nc.tensor.matmul(ps[A * ci:A * (ci + 1), :],
                                         W("qw", A), act[:, bass.ts(c, 512)],
                                         start=True, stop=True,
                                         skip_group_check=True,
                                         tile_position=(0, A * ci))
                    o_sb = out_pool.tile([128, 512], F32, tag="o")
                    nc.vector.tensor_scalar(out=o_sb[:m, :], in0=ps[:m, :],
                                            scalar1=Bi("qb")[:m, :],
                                            scalar2=None, op0=OP.add)
                    for ci in range(nq):
                        c = c0 + ci
                        nc.sync.dma_start(
                            out=y_d.ap()[:, r0 + 512 * c:r0 + 512 * (c + 1)],
                            in_=o_sb[A * ci:A * (ci + 1), :])

    nc.compile()
    return nc


def _bf16(a):
    import ml_dtypes
    return np.asarray(a, np.float32).astype(ml_dtypes.bfloat16)


def pack_weights(inputs):
    """Build the bf16 weight blob [128, WCOLS] and fp32 bias blob."""
    wb = np.zeros((128, WCOLS), np.float32)
    ew = np.asarray(inputs["enc_w"], np.float32)          # [256, 128]
    wb[:, 0:128] = ew[:128]
    wb[:, 128:256] = ew[128:]
    for l in ("a1", "a2"):
        for p in ("q", "k", "v", "o"):
            wb[:, WOFF[f"{l}_{p}"]:WOFF[f"{l}_{p}"] + H] = np.asarray(
                inputs[f"{l}_{p}w"], np.float32)
    wb[:, WOFF["qw"]:WOFF["qw"] + A] = np.asarray(inputs["q_w"], np.float32)
    bbl = np.zeros((128, len(BOFF)), np.float32)
    bbl[:, 0] = inputs["enc_b"]
    for l in ("a1", "a2"):
        for i, p in enumerate(("q", "k", "v", "o")):
            bbl[:, BOFF[f"{l}_{p}"]] = inputs[f"{l}_{p}b"]
    bbl[:, BOFF["qb"]] = np.tile(np.asarray(inputs["q_b"], np.float32), 4)
    return _bf16(wb), bbl


def prep_inputs_core(x_c, mask_c):
    """Per-core: x -> bf16 [DIN, rows]; mask -> (1-met) bf16 block tiles."""
    rows = x_c.shape[0] * N
    xt = _bf16(np.ascontiguousarray(
        np.asarray(x_c, np.float32).reshape(rows, DIN).T))
    ng = x_c.shape[0] // 4
    m4 = np.asarray(mask_c, np.float32).reshape(ng, 4, N, N)
    met = np.zeros((ng, 128, 128), np.float32)
    for b in range(4):
        met[:, 32 * b:32 * b + 32, 32 * b:32 * b + 32] = m4[:, b]
    mc = np.ascontiguousarray(met.transpose(1, 0, 2))      # [128, G, 128]
    return xt, _bf16(mc)


_CACHE = {}


def build_in_maps(inputs):
    x, mask = inputs["x"], inputs["mask"]
    wb, bbl = pack_weights(inputs)
    in_maps = []
    for c in range(NCORES):
        xt, mc = prep_inputs_core(x[c * BC:(c + 1) * BC],
                                  mask[c * BC:(c + 1) * BC])
        in_maps.append({"xt": xt, "metc": mc, "wblob": wb, "bblob": bbl})
    return in_maps


def kernel(**inputs):
    if "nc" not in _CACHE:
        _CACHE["nc"] = build_program(R)
    res = run_bass_kernel_spmd(_CACHE["nc"], build_in_maps(inputs),
                               core_ids=list(range(NCORES)))
    outs = [r["y"].T.reshape(BC, N, A) for r in res.results]
    return np.concatenate(outs, axis=0).astype(np.float32)

